# revision 47
# baseline (speedup 1.0000x reference)
"""Causal self-attention (QKV projection + softmax(QK^T/sqrt(N)) @ V) on 8 TRN2
NeuronCores.

Sharding: core c = 2*b + j handles batch element b (of 4) and half the query
rows, as four 256-row query blocks balanced across the causal triangle
(j=0: blocks {0,3,4,7}, j=1: {1,2,5,6} of the eight 256-row blocks). At
schedule position p (0..3) every core's block needs at most 4*(p+1) key tiles
of 128, so a single uniform SPMD program computes KT_p = 4*(p+1) key tiles per
position and per-core masks (built from shipped position vectors) make it
correct — 40 key-tile iterations per core vs 68 for an exact causal split and
96 for the naive half/half split. The key side needs NO permutation; only the
query side (ctxQ columns, qpos, output rows) is per-core.

The kernel never materializes K or V. Both big projections are reassociated so
per-core work scales with the core's OWN 1024 queries instead of the full
2048-key sequence (which is duplicated across the core pair):

  scores = (ctx Wk + bk)(ctx Wq + bq)^T
         = ctx (Wk Wq^T) ctx^T + a_k + (q-terms that cancel in softmax)
    -> host folds WkqT = (Wq Wk^T)/sqrt(N) (weight-only), device computes
       U = WkqT^T ctx_q^T per query block, then S^T = ctx^T-tiles.T @ U per
       key tile; a_k = ctx (Wk bq)/sqrt(N) is a host matvec shipped as a
       per-key-tile activation bias for the Exp.
  out   = P (ctx Wv + bv) = (P^T ctx) Wv + bv   (sum P = 1 after normalize)
    -> device computes op^T = ctx_rows-tiles.T @ P per d-tile, then
       out = op Wv / den (+ bv).

Per-core matmuls: 256 U + 320 S + 320 PV (all 256-free, full rate ~109ns) +
128 Wv (512-free, ~216ns) + 80 small denominators. All operands bf16 with f32
PSUM; simulated end-to-end rel err ~4e-3 vs the 2e-2 gate. Everything is
SBUF-resident (~19MB); the only HBM traffic after the input stream is the
output writes.
"""

import math
from contextlib import ExitStack

import numpy as np

import concourse.bass as bass
import concourse.mybir as mybir
import concourse.tile as tile
from concourse.bass_utils import run_bass_kernel_spmd

P = 128
CH = 512   # ctx chunk columns (key side)
QB = 256   # query block rows
NPOS = 4   # query blocks per core


def _fix_matmul_waits(nc):
    """Walrus codegen has a small per-instruction sync-wait slot budget (one
    for a self-loading matmul's LDWEIGHTS half, similar for ACT etc). Move
    extra waits onto NoOps inserted just before the instruction on the same
    engine — per-engine program order (and thus semantics) is unchanged."""
    skip = (mybir.InstEventSemaphore, mybir.InstNoOp,
            mybir.InstUnconditionalBranch, mybir.InstCall)
    for func in nc.m.functions:
        for bb in func.blocks:
            il = bb.instructions
            new = []
            changed = False
            for inst in il:
                si = getattr(inst, "sync_info", None)
                if (si and si.on_wait and len(si.on_wait) > 1
                        and not isinstance(inst, skip)):
                    waits = list(si.on_wait)
                    for wi, w in enumerate(waits[:-1]):
                        nop = mybir.InstNoOp(
                            name=f"{inst.name}-wfix{wi}", engine=inst.engine,
                            sync_info=mybir.SyncInfo(on_wait=[w], on_update=[]),
                            text_hint="waitfix")
                        new.append(nop)
                    inst.sync_info = mybir.SyncInfo(
                        on_wait=[waits[-1]], on_update=list(si.on_update or []))
                    changed = True
                new.append(inst)
            if changed:
                bb.instructions = new


def build(N=2048, D=1024, has_bias=True, fix_waits=True, **bass_kwargs):
    NT = N // P          # key tiles
    DN = D // P          # 128-blocks of the model dim
    NCH = N // CH        # key-side ctx chunks
    QTOT = NPOS * QB     # query rows per core
    QT = QB // P         # 128-row groups per query block
    BF = mybir.dt.bfloat16
    F32 = mybir.dt.float32
    AF = mybir.ActivationFunctionType
    OP = mybir.AluOpType

    nc = bass.Bass(**bass_kwargs)

    CW = QTOT + 2 * NT + D  # packed f32 consts: qpos | kpos | abk | bvb
    # all inputs ship as host-prepacked SBUF images -> plain contiguous 2D DMAs
    ctk_d = nc.declare_dram_parameter("ctk_d", [P, NCH * DN * CH], BF, isOutput=False)
    ctr_d = nc.declare_dram_parameter("ctr_d", [P, NT * D], BF, isOutput=False)
    ctq_d = nc.declare_dram_parameter("ctq_d", [P, NPOS * DN * QB], BF, isOutput=False)
    wkq_d = nc.declare_dram_parameter("wkq_d", [P, D * DN], BF, isOutput=False)
    wv_d = nc.declare_dram_parameter("wv_d", [P, D * DN], BF, isOutput=False)
    cstd = nc.declare_dram_parameter("cstd", [P, CW], F32, isOutput=False)
    onesd = nc.declare_dram_parameter("onesd", [P, P], BF, isOutput=False)
    out_ext = nc.declare_dram_parameter("out", [QTOT, D], BF, isOutput=True)

    with ExitStack() as ctx:
        tc = ctx.enter_context(tile.TileContext(nc))
        const = ctx.enter_context(tc.tile_pool(name="const", bufs=1))
        persist = ctx.enter_context(tc.tile_pool(name="persist", bufs=1))
        mpool = ctx.enter_context(tc.tile_pool(name="mp", bufs=3))
        rpool = ctx.enter_context(tc.tile_pool(name="rp", bufs=1))
        opool = ctx.enter_context(tc.tile_pool(name="op", bufs=3))
        pp = ctx.enter_context(tc.tile_pool(name="pp", bufs=4, space="PSUM"))

        # SBUF stores (bf16), layouts produced host-side:
        #   wkq_sb: lhsT tile (t_out, c_contract) at col (t*DN+c)*P      (2MB)
        #   ctk_sb: ctx^T key side, col ch*DN*CH + d*CH + token          (4MB)
        #   ctr_sb: ctx rows, col kt*D + d                               (4MB)
        #   ctq_sb: query ctx^T, col pos*DN*QB + d*QB + q                (2MB)
        #   u_store: col t*QTOT + pos*QB + q                             (2MB)
        #   e_all: col kt*QB + q (per-position scratch)                  (1MB)
        #   op_sb: col d*QB + q (per-position scratch)                 (0.5MB)
        wkq_sb = persist.tile([P, D * DN], BF, name="wkq")
        wv_sb = persist.tile([P, D * DN], BF, name="wv")
        ctk_sb = persist.tile([P, NCH * DN * CH], BF, name="ctk")
        cts = [ctk_sb[:, ch * DN * CH:(ch + 1) * DN * CH] for ch in range(NCH)]
        ctr_sb = persist.tile([P, NT * D], BF, name="ctr")
        ctq_sb = persist.tile([P, NPOS * DN * QB], BF, name="ctq")
        u_store = persist.tile([P, DN * QTOT], BF, name="u_store")
        e_all = persist.tile([P, NT * QB], BF, name="e_all")
        op_sb = persist.tile([P, DN * QB], BF, name="op_sb")

        cst_sb = const.tile([P, CW], F32)
        qpos_sb = cst_sb[:, 0:QTOT]
        kpos_sb = cst_sb[:, QTOT:QTOT + NT]
        ab_sb = cst_sb[:, QTOT + NT:QTOT + 2 * NT]
        bv_sb = cst_sb[:, QTOT + 2 * NT:]
        ones_sb = const.tile([P, P], BF)

        # ---- DMA stream: plain-2D transfers in exact consumption order.
        # Two tiny starter pieces let the first U matmul begin ~4us earlier;
        # wkq streams per t-block so the U t-loop never outruns the ring.
        def ld2(dst, src, lo, hi):
            nc.sync.dma_start(out=dst[:, lo:hi], in_=src[:, lo:hi])

        ld2(wkq_sb, wkq_d, 0, P)               # starter: lhsT tile (t0,c0)
        ld2(ctq_sb, ctq_d, 0, QB)              # starter: rhs block (pos0,c0)
        ld2(wkq_sb, wkq_d, P, D)               # rest of t0
        ld2(ctq_sb, ctq_d, QB, DN * QB)        # rest of pos0
        for t in range(1, 4):
            ld2(wkq_sb, wkq_d, t * D, (t + 1) * D)
        ld2(ctq_sb, ctq_d, DN * QB, 2 * DN * QB)      # pos1
        for t in range(4, DN):
            ld2(wkq_sb, wkq_d, t * D, (t + 1) * D)
        ld2(ctq_sb, ctq_d, 2 * DN * QB, NPOS * DN * QB)
        nc.sync.dma_start(out=ctk_sb, in_=ctk_d[:, :])
        nc.sync.dma_start(out=cst_sb, in_=cstd[:, :])
        nc.sync.dma_start(out=ones_sb, in_=onesd[:, :])
        nc.sync.dma_start(out=ctr_sb, in_=ctr_d[:, :])
        nc.sync.dma_start(out=wv_sb, in_=wv_d[:, :])

        # ---- U = WkqT^T @ ctx_q^T for all four query blocks ----
        for pos in range(NPOS):
            for t in range(DN):
                psu = pp.tile([P, CH], F32, tag="big", name="psu")
                for c in range(DN):
                    nc.tensor.matmul(
                        psu[:, :QB],
                        lhsT=wkq_sb[:, (t * DN + c) * P:(t * DN + c + 1) * P],
                        rhs=ctq_sb[:, pos * DN * QB + c * QB:pos * DN * QB + (c + 1) * QB],
                        start=(c == 0), stop=(c == DN - 1))
                nc.scalar.activation(
                    u_store[:, t * QTOT + pos * QB:t * QTOT + (pos + 1) * QB],
                    psu[:, :QB], AF.Copy)

        # ---- attention, one 256-row query block per schedule position ----
        for pos in range(NPOS):
            KT = 4 * (pos + 1)
            # scores + exp + mask (top 4 key tiles of each position)
            for k in range(KT):
                ch, loc = divmod(k, CH // P)
                pss = pp.tile([P, CH], F32, tag="big", name="pss")
                for d in range(DN):
                    nc.tensor.matmul(
                        pss[:, :QB],
                        lhsT=cts[ch][:, d * CH + loc * P:d * CH + (loc + 1) * P],
                        rhs=u_store[:, d * QTOT + pos * QB:d * QTOT + (pos + 1) * QB],
                        start=(d == 0), stop=(d == DN - 1))
                esl = e_all[:, k * QB:(k + 1) * QB]
                if has_bias:
                    nc.scalar.activation(esl, pss[:, :QB], AF.Exp, bias=ab_sb[:, k:k + 1])
                else:
                    nc.scalar.activation(esl, pss[:, :QB], AF.Exp)
                if k >= KT - 4:
                    m = mpool.tile([P, QB], BF, tag="m", name="m")
                    nc.vector.tensor_scalar(m, qpos_sb[:, pos * QB:(pos + 1) * QB],
                                            kpos_sb[:, k:k + 1], None, OP.is_ge)
                    nc.vector.tensor_tensor(esl, esl, m, OP.mult)
            # denominator via a ones-lhsT PV iteration: every output partition
            # row holds den[q], i.e. den lands in q-FREE orientation so op_sb
            # (also q-free) can be normalized directly by the vector engine.
            # Runs first so rec_bcast is ready while the d-loop computes.
            # (kt-loop waits only per-kt masks, never the last one.)
            pde = pp.tile([P, CH], F32, tag="big", name="pde")
            for k in range(KT):
                nc.tensor.matmul(
                    pde[:, :QB], lhsT=ones_sb, rhs=e_all[:, k * QB:(k + 1) * QB],
                    start=(k == 0), stop=(k == KT - 1))
            rec_b = rpool.tile([P, QB], F32, tag="recb", name="rec_b")
            nc.vector.reciprocal(rec_b, pde[:, :QB])
            # op^T = ctx_rows-tiles.T @ P (one d-tile / PSUM bank at a time),
            # normalized in place right after each eviction
            for d in range(DN):
                ppv = pp.tile([P, CH], F32, tag="big", name="ppv")
                for k in range(KT):
                    nc.tensor.matmul(
                        ppv[:, :QB], lhsT=ctr_sb[:, k * D + d * P:k * D + (d + 1) * P],
                        rhs=e_all[:, k * QB:(k + 1) * QB],
                        start=(k == 0), stop=(k == KT - 1))
                osl = op_sb[:, d * QB:(d + 1) * QB]
                nc.scalar.activation(osl, ppv[:, :QB], AF.Copy)
                nc.vector.tensor_tensor(osl, osl, rec_b, OP.mult)
            # out = op Wv / den (+ bv)
            for qt in range(QT):
                for ei in range(D // CH):
                    psf = pp.tile([P, CH], F32, tag="big", name="psf")
                    for d in range(DN):
                        nc.tensor.matmul(
                            psf, lhsT=op_sb[:, d * QB + qt * P:d * QB + (qt + 1) * P],
                            rhs=wv_sb[:, d * D + ei * CH:d * D + (ei + 1) * CH],
                            start=(d == 0), stop=(d == DN - 1))
                    ot = opool.tile([P, CH], BF, tag="o", name="ot")
                    if has_bias:
                        nc.vector.tensor_tensor(ot, psf, bv_sb[:, ei * CH:(ei + 1) * CH], OP.add)
                    else:
                        # op was pre-normalized; plain PSUM->SBUF copy on the
                        # same engine as the out DMA (no cross-engine hop)
                        nc.scalar.activation(ot, psf, AF.Copy)
                    nc.scalar.dma_start(
                        out=out_ext[pos * QB + qt * P:pos * QB + (qt + 1) * P,
                                    ei * CH:(ei + 1) * CH],
                        in_=ot)
    if fix_waits:
        _fix_matmul_waits(nc)
    return nc


def _block_order(j):
    # four 256-row blocks per core, position p needing <= 4*(p+1) key tiles
    return [0, 3, 4, 7] if j == 0 else [1, 2, 5, 6]


def make_in_maps(context, W_qkv, b_qkv, n_cores=8):
    import ml_dtypes
    bf16 = ml_dtypes.bfloat16
    context = np.ascontiguousarray(np.asarray(context, np.float32))
    W_qkv = np.asarray(W_qkv, np.float32)
    b_qkv = np.asarray(b_qkv, np.float32)
    B, N, D = context.shape
    NT = N // P
    QTOT = NPOS * QB
    SCALE = 1.0 / math.sqrt(N)
    Wq, Wk, Wv = W_qkv[:, :D], W_qkv[:, D:2 * D], W_qkv[:, 2 * D:]
    bq, bk, bv = b_qkv[:D], b_qkv[D:2 * D], b_qkv[2 * D:]
    DN = D // P
    NPOSL = NPOS
    wkqT = ((Wq @ Wk.T) * SCALE).astype(bf16)          # [D(c-rows), D(t-cols)]
    # wkq SBUF image: col (t*DN+c)*P + x  <-  wkqT[c*P+p, t*P+x]
    wkq_img = np.ascontiguousarray(
        wkqT.reshape(DN, P, DN, P).transpose(1, 2, 0, 3).reshape(P, D * DN))
    # wv SBUF image: col d*D + e  <-  Wv[d*P+p, e]
    wv_img = np.ascontiguousarray(
        Wv.astype(bf16).reshape(DN, P, D).transpose(1, 0, 2).reshape(P, D * DN))
    bvb = np.broadcast_to(bv, (P, D)).astype(np.float32)
    wkbq = (Wk @ bq) * SCALE  # [D]; a_k = ctx_k . wkbq (k-dependent exp bias)
    kpos = (np.arange(NT)[None, :] * P + np.arange(P)[:, None]).astype(np.float32)
    in_maps = []
    for c in range(n_cores):
        b, j = divmod(c, 2)
        order = _block_order(j)
        ctx_b = context[b]
        ctx_bT = ctx_b.T.astype(bf16)                   # [D, N]
        # key-side image: col ch*DN*CH + d*CH + tok  <-  ctxT[d*P+p, ch*CH+tok]
        ctk_img = np.ascontiguousarray(
            ctx_bT.reshape(DN, P, N // CH, CH).transpose(1, 2, 0, 3).reshape(P, -1))
        # ctx-rows image: col kt*D + dd  <-  ctx[kt*P+p, dd]
        ctr_img = np.ascontiguousarray(
            ctx_b.astype(bf16).reshape(NT, P, D).transpose(1, 0, 2).reshape(P, -1))
        # query image: col pos*DN*QB + d*QB + q  <-  ctxT[d*P+p, block_q]
        ctxQ = np.concatenate(
            [ctx_bT[:, o * QB:(o + 1) * QB] for o in order], axis=1)  # [D, QTOT]
        ctq_img = np.ascontiguousarray(
            ctxQ.reshape(DN, P, NPOSL, QB).transpose(1, 2, 0, 3).reshape(P, -1))
        qpos_row = np.concatenate(
            [np.arange(o * QB, (o + 1) * QB) for o in order]).astype(np.float32)
        qpos_b = np.broadcast_to(qpos_row, (P, QTOT))
        a_full = (ctx_b @ wkbq).astype(np.float32)  # [N]
        abk = a_full.reshape(NT, P).T
        cst = np.ascontiguousarray(  # qpos | kpos | abk | bvb
            np.concatenate([qpos_b, kpos, abk, bvb], axis=1, dtype=np.float32))
        in_maps.append({
            "ctk_d": ctk_img, "ctr_d": ctr_img, "ctq_d": ctq_img,
            "wkq_d": wkq_img, "wv_d": wv_img,
            "cstd": cst,
            "onesd": np.ones((P, P), bf16),
        })
    return in_maps


def assemble(results, B, N, D):
    out = np.zeros((B, N, D), np.float32)
    for c, res in enumerate(results):
        b, j = divmod(c, 2)
        order = _block_order(j)
        o = np.asarray(res["out"], np.float32)
        for p, blk in enumerate(order):
            out[b, blk * QB:(blk + 1) * QB] = o[p * QB:(p + 1) * QB]
    return out


def run(inputs, trace=False, **spmd_kwargs):
    context = np.asarray(inputs["context"])
    B, N, D = context.shape
    has_bias = bool(np.any(np.asarray(inputs["b_qkv"])))
    nc = build(N, D, has_bias=has_bias)
    in_maps = make_in_maps(context, inputs["W_qkv"], inputs["b_qkv"], n_cores=8)
    res = run_bass_kernel_spmd(nc, in_maps, core_ids=list(range(8)), trace=trace, **spmd_kwargs)
    out = assemble(res.results, B, N, D)
    return out, res


def kernel(context, W_qkv, b_qkv):
    out, _ = run({"context": context, "W_qkv": W_qkv, "b_qkv": b_qkv})
    return out


# revision 50
# speedup vs baseline: 1.0364x; 1.0364x over previous
"""Causal self-attention (QKV projection + softmax(QK^T/sqrt(N)) @ V) on 8 TRN2
NeuronCores.

Sharding: core c = 2*b + j handles batch element b (of 4) and half the query
rows, as four 256-row query blocks balanced across the causal triangle
(j=0: blocks {0,3,4,7}, j=1: {1,2,5,6} of the eight 256-row blocks). At
schedule position p (0..3) every core's block needs at most 4*(p+1) key tiles
of 128, so a single uniform SPMD program computes KT_p = 4*(p+1) key tiles per
position and per-core masks (built from shipped position vectors) make it
correct — 40 key-tile iterations per core vs 68 for an exact causal split and
96 for the naive half/half split. The key side needs NO permutation; only the
query side (ctxQ columns, qpos, output rows) is per-core.

The kernel never materializes K or V. Both big projections are reassociated so
per-core work scales with the core's OWN 1024 queries instead of the full
2048-key sequence (which is duplicated across the core pair):

  scores = (ctx Wk + bk)(ctx Wq + bq)^T
         = ctx (Wk Wq^T) ctx^T + a_k + (q-terms that cancel in softmax)
    -> host folds WkqT = (Wq Wk^T)/sqrt(N) (weight-only), device computes
       U = WkqT^T ctx_q^T per query block, then S^T = ctx^T-tiles.T @ U per
       key tile; a_k = ctx (Wk bq)/sqrt(N) is a host matvec shipped as a
       per-key-tile activation bias for the Exp.
  out   = P (ctx Wv + bv) = (P^T ctx) Wv + bv   (sum P = 1 after normalize)
    -> device computes op^T = ctx_rows-tiles.T @ P per d-tile, then
       out = op Wv / den (+ bv).

Per-core matmuls: 256 U + 320 S + 320 PV (all 256-free, full rate ~109ns) +
128 Wv (512-free, ~216ns) + 80 small denominators. All operands bf16 with f32
PSUM; simulated end-to-end rel err ~4e-3 vs the 2e-2 gate. Everything is
SBUF-resident (~19MB); the only HBM traffic after the input stream is the
output writes.
"""

import math
from contextlib import ExitStack

import numpy as np

import concourse.bass as bass
import concourse.mybir as mybir
import concourse.tile as tile
from concourse.bass_utils import run_bass_kernel_spmd

P = 128
CH = 512   # ctx chunk columns (key side)
QB = 256   # query block rows
NPOS = 4   # query blocks per core


def _fix_matmul_waits(nc):
    """Walrus codegen has a small per-instruction sync-wait slot budget (one
    for a self-loading matmul's LDWEIGHTS half, similar for ACT etc). Move
    extra waits onto NoOps inserted just before the instruction on the same
    engine — per-engine program order (and thus semantics) is unchanged."""
    skip = (mybir.InstEventSemaphore, mybir.InstNoOp,
            mybir.InstUnconditionalBranch, mybir.InstCall)
    for func in nc.m.functions:
        for bb in func.blocks:
            il = bb.instructions
            new = []
            changed = False
            for inst in il:
                si = getattr(inst, "sync_info", None)
                if (si and si.on_wait and len(si.on_wait) > 1
                        and not isinstance(inst, skip)):
                    waits = list(si.on_wait)
                    for wi, w in enumerate(waits[:-1]):
                        nop = mybir.InstNoOp(
                            name=f"{inst.name}-wfix{wi}", engine=inst.engine,
                            sync_info=mybir.SyncInfo(on_wait=[w], on_update=[]),
                            text_hint="waitfix")
                        new.append(nop)
                    inst.sync_info = mybir.SyncInfo(
                        on_wait=[waits[-1]], on_update=list(si.on_update or []))
                    changed = True
                new.append(inst)
            if changed:
                bb.instructions = new


def build(N=2048, D=1024, has_bias=True, fix_waits=True, **bass_kwargs):
    NT = N // P          # key tiles
    DN = D // P          # 128-blocks of the model dim
    NCH = N // CH        # key-side ctx chunks
    QTOT = NPOS * QB     # query rows per core
    QT = QB // P         # 128-row groups per query block
    BF = mybir.dt.bfloat16
    F32 = mybir.dt.float32
    AF = mybir.ActivationFunctionType
    OP = mybir.AluOpType

    nc = bass.Bass(**bass_kwargs)

    CW = QTOT + 2 * NT + D  # packed f32 consts: qpos | kpos | abk | bvb
    # all inputs ship as host-prepacked SBUF images -> plain contiguous 2D DMAs
    ctk_d = nc.declare_dram_parameter("ctk_d", [P, NCH * DN * CH], BF, isOutput=False)
    ctr_d = nc.declare_dram_parameter("ctr_d", [P, NT * D], BF, isOutput=False)
    ctq_d = nc.declare_dram_parameter("ctq_d", [P, NPOS * DN * QB], BF, isOutput=False)
    wkq_d = nc.declare_dram_parameter("wkq_d", [P, D * DN], BF, isOutput=False)
    wv_d = nc.declare_dram_parameter("wv_d", [P, D * DN], BF, isOutput=False)
    cstd = nc.declare_dram_parameter("cstd", [P, CW], F32, isOutput=False)
    onesd = nc.declare_dram_parameter("onesd", [P, 8], BF, isOutput=False)
    out_ext = nc.declare_dram_parameter("out", [QTOT, D], BF, isOutput=True)

    with ExitStack() as ctx:
        tc = ctx.enter_context(tile.TileContext(nc))
        const = ctx.enter_context(tc.tile_pool(name="const", bufs=1))
        persist = ctx.enter_context(tc.tile_pool(name="persist", bufs=1))
        mpool = ctx.enter_context(tc.tile_pool(name="mp", bufs=3))
        rpool = ctx.enter_context(tc.tile_pool(name="rp", bufs=1))
        opool = ctx.enter_context(tc.tile_pool(name="op", bufs=3))
        pp = ctx.enter_context(tc.tile_pool(name="pp", bufs=4, space="PSUM"))
        dpp = ctx.enter_context(tc.tile_pool(name="dpp", bufs=1, space="PSUM"))

        # SBUF stores (bf16), layouts produced host-side:
        #   wkq_sb: lhsT tile (t_out, c_contract) at col (t*DN+c)*P      (2MB)
        #   ctk_sb: ctx^T key side, col ch*DN*CH + d*CH + token          (4MB)
        #   ctr_sb: ctx rows, col kt*D + d                               (4MB)
        #   ctq_sb: query ctx^T, col pos*DN*QB + d*QB + q                (2MB)
        #   u_store: col t*QTOT + pos*QB + q                             (2MB)
        #   e_all: col kt*QB + q (per-position scratch)                  (1MB)
        #   op_sb: col d*QB + q (per-position scratch)                 (0.5MB)
        wkq_sb = persist.tile([P, D * DN], BF, name="wkq")
        wv_sb = persist.tile([P, D * DN], BF, name="wv")
        ctk_sb = persist.tile([P, NCH * DN * CH], BF, name="ctk")
        cts = [ctk_sb[:, ch * DN * CH:(ch + 1) * DN * CH] for ch in range(NCH)]
        ctr_sb = persist.tile([P, NT * D], BF, name="ctr")
        ctq_sb = persist.tile([P, NPOS * DN * QB], BF, name="ctq")
        u_store = persist.tile([P, DN * QTOT], BF, name="u_store")
        e_all = persist.tile([P, 28 * QB], BF, name="e_all")
        op_sb = persist.tile([P, DN * QB], BF, name="op_sb")

        cst_sb = const.tile([P, CW], F32)
        qpos_sb = cst_sb[:, 0:QTOT]
        kpos_sb = cst_sb[:, QTOT:QTOT + NT]
        ab_sb = cst_sb[:, QTOT + NT:QTOT + 2 * NT]
        bv_sb = cst_sb[:, QTOT + 2 * NT:]
        ones_sb = const.tile([P, 8], BF)

        # ---- DMA stream: plain-2D transfers in exact consumption order.
        # Two tiny starter pieces let the first U matmul begin ~4us earlier;
        # wkq streams per t-block so the U t-loop never outruns the ring.
        def ld2(dst, src, lo, hi):
            nc.sync.dma_start(out=dst[:, lo:hi], in_=src[:, lo:hi])

        ld2(wkq_sb, wkq_d, 0, P)               # starter: lhsT tile (t0,c0)
        ld2(ctq_sb, ctq_d, 0, QB)              # starter: rhs block (pos0,c0)
        ld2(wkq_sb, wkq_d, P, D)               # rest of t0
        ld2(ctq_sb, ctq_d, QB, DN * QB)        # rest of pos0
        for t in range(1, 4):
            ld2(wkq_sb, wkq_d, t * D, (t + 1) * D)
        ld2(ctq_sb, ctq_d, DN * QB, 2 * DN * QB)      # pos1
        for t in range(4, DN):
            ld2(wkq_sb, wkq_d, t * D, (t + 1) * D)
        ld2(ctq_sb, ctq_d, 2 * DN * QB, NPOS * DN * QB)
        nc.sync.dma_start(out=ctk_sb, in_=ctk_d[:, :])
        nc.sync.dma_start(out=cst_sb, in_=cstd[:, :])
        nc.sync.dma_start(out=ones_sb, in_=onesd[:, :])
        nc.sync.dma_start(out=ctr_sb, in_=ctr_d[:, :])
        nc.sync.dma_start(out=wv_sb, in_=wv_d[:, :])

        # ---- U = WkqT^T @ ctx_q^T for all four query blocks ----
        for pos in range(NPOS):
            for t in range(DN):
                psu = pp.tile([P, CH], F32, tag="big", name="psu")
                for c in range(DN):
                    nc.tensor.matmul(
                        psu[:, :QB],
                        lhsT=wkq_sb[:, (t * DN + c) * P:(t * DN + c + 1) * P],
                        rhs=ctq_sb[:, pos * DN * QB + c * QB:pos * DN * QB + (c + 1) * QB],
                        start=(c == 0), stop=(c == DN - 1))
                nc.scalar.activation(
                    u_store[:, t * QTOT + pos * QB:t * QTOT + (pos + 1) * QB],
                    psu[:, :QB], AF.Copy)

        # ---- attention, one 256-row query block per schedule position ----
        # Positions run in PAIRS: both scores loops emit back-to-back, then
        # both psd/PV/Wv groups. The second scores loop covers the first
        # position's exp->mask latency, so no consumer ever stalls on a mask.
        # Each pair uses disjoint e_all slot ranges (pair peaks at 4+8 / 12+16).
        EBASE = {0: 0, 1: 4, 2: 0, 3: 12}
        for pair in ((0, 1), (2, 3)):
            for pos in pair:
                KT = 4 * (pos + 1)
                eb = EBASE[pos]
                # scores + exp + mask (top 4 key tiles of each position)
                for k in range(KT):
                    ch, loc = divmod(k, CH // P)
                    pss = pp.tile([P, CH], F32, tag="big", name="pss")
                    for d in range(DN):
                        nc.tensor.matmul(
                            pss[:, :QB],
                            lhsT=cts[ch][:, d * CH + loc * P:d * CH + (loc + 1) * P],
                            rhs=u_store[:, d * QTOT + pos * QB:d * QTOT + (pos + 1) * QB],
                            start=(d == 0), stop=(d == DN - 1))
                    esl = e_all[:, (eb + k) * QB:(eb + k + 1) * QB]
                    if has_bias:
                        nc.scalar.activation(esl, pss[:, :QB], AF.Exp, bias=ab_sb[:, k:k + 1])
                    else:
                        nc.scalar.activation(esl, pss[:, :QB], AF.Exp)
                    if k >= KT - 4:
                        m = mpool.tile([P, QB], BF, tag="m", name="m")
                        nc.vector.tensor_scalar(m, qpos_sb[:, pos * QB:(pos + 1) * QB],
                                                kpos_sb[:, k:k + 1], None, OP.is_ge)
                        nc.vector.tensor_tensor(esl, esl, m, OP.mult)
            for pos in pair:
                KT = 4 * (pos + 1)
                eb = EBASE[pos]
                # denominators + reciprocals
                psd = [dpp.tile([P, 8], F32, tag=f"den{qt}", name="psd") for qt in range(QT)]
                for qt in range(QT):
                    for k in range(KT):
                        nc.tensor.matmul(
                            psd[qt],
                            lhsT=e_all[:, (eb + k) * QB + qt * P:(eb + k) * QB + (qt + 1) * P],
                            rhs=ones_sb, start=(k == 0), stop=(k == KT - 1))
                recs = []
                for qt in range(QT):
                    rec = rpool.tile([P, 1], F32, tag=f"rec{qt}", name="rec")
                    nc.vector.reciprocal(rec, psd[qt][:, 0:1])
                    recs.append(rec)
                # op^T = ctx_rows-tiles.T @ P (one d-tile / PSUM bank at a time)
                for d in range(DN):
                    ppv = pp.tile([P, CH], F32, tag="big", name="ppv")
                    for k in range(KT):
                        nc.tensor.matmul(
                            ppv[:, :QB], lhsT=ctr_sb[:, k * D + d * P:k * D + (d + 1) * P],
                            rhs=e_all[:, (eb + k) * QB:(eb + k + 1) * QB],
                            start=(k == 0), stop=(k == KT - 1))
                    nc.scalar.activation(op_sb[:, d * QB:(d + 1) * QB], ppv[:, :QB], AF.Copy)
                # out = op Wv / den (+ bv)
                for qt in range(QT):
                    for ei in range(D // CH):
                        psf = pp.tile([P, CH], F32, tag="big", name="psf")
                        for d in range(DN):
                            nc.tensor.matmul(
                                psf, lhsT=op_sb[:, d * QB + qt * P:d * QB + (qt + 1) * P],
                                rhs=wv_sb[:, d * D + ei * CH:d * D + (ei + 1) * CH],
                                start=(d == 0), stop=(d == DN - 1))
                        ot = opool.tile([P, CH], BF, tag="o", name="ot")
                        if has_bias:
                            nc.vector.tensor_scalar_mul(ot, psf, recs[qt])
                            nc.vector.tensor_tensor(ot, ot, bv_sb[:, ei * CH:(ei + 1) * CH], OP.add)
                        else:
                            # normalize on Scalar: same engine as the out DMA,
                            # so the tail has no cross-engine hop
                            nc.scalar.activation(ot, psf, AF.Copy, scale=recs[qt])
                        nc.scalar.dma_start(
                            out=out_ext[pos * QB + qt * P:pos * QB + (qt + 1) * P,
                                        ei * CH:(ei + 1) * CH],
                            in_=ot)
    if fix_waits:
        _fix_matmul_waits(nc)
    return nc


def _block_order(j):
    # four 256-row blocks per core, position p needing <= 4*(p+1) key tiles
    return [0, 3, 4, 7] if j == 0 else [1, 2, 5, 6]


def make_in_maps(context, W_qkv, b_qkv, n_cores=8):
    import ml_dtypes
    bf16 = ml_dtypes.bfloat16
    context = np.ascontiguousarray(np.asarray(context, np.float32))
    W_qkv = np.asarray(W_qkv, np.float32)
    b_qkv = np.asarray(b_qkv, np.float32)
    B, N, D = context.shape
    NT = N // P
    QTOT = NPOS * QB
    SCALE = 1.0 / math.sqrt(N)
    Wq, Wk, Wv = W_qkv[:, :D], W_qkv[:, D:2 * D], W_qkv[:, 2 * D:]
    bq, bk, bv = b_qkv[:D], b_qkv[D:2 * D], b_qkv[2 * D:]
    DN = D // P
    NPOSL = NPOS
    wkqT = ((Wq @ Wk.T) * SCALE).astype(bf16)          # [D(c-rows), D(t-cols)]
    # wkq SBUF image: col (t*DN+c)*P + x  <-  wkqT[c*P+p, t*P+x]
    wkq_img = np.ascontiguousarray(
        wkqT.reshape(DN, P, DN, P).transpose(1, 2, 0, 3).reshape(P, D * DN))
    # wv SBUF image: col d*D + e  <-  Wv[d*P+p, e]
    wv_img = np.ascontiguousarray(
        Wv.astype(bf16).reshape(DN, P, D).transpose(1, 0, 2).reshape(P, D * DN))
    bvb = np.broadcast_to(bv, (P, D)).astype(np.float32)
    wkbq = (Wk @ bq) * SCALE  # [D]; a_k = ctx_k . wkbq (k-dependent exp bias)
    kpos = (np.arange(NT)[None, :] * P + np.arange(P)[:, None]).astype(np.float32)
    in_maps = []
    for c in range(n_cores):
        b, j = divmod(c, 2)
        order = _block_order(j)
        ctx_b = context[b]
        ctx_bT = ctx_b.T.astype(bf16)                   # [D, N]
        # key-side image: col ch*DN*CH + d*CH + tok  <-  ctxT[d*P+p, ch*CH+tok]
        ctk_img = np.ascontiguousarray(
            ctx_bT.reshape(DN, P, N // CH, CH).transpose(1, 2, 0, 3).reshape(P, -1))
        # ctx-rows image: col kt*D + dd  <-  ctx[kt*P+p, dd]
        ctr_img = np.ascontiguousarray(
            ctx_b.astype(bf16).reshape(NT, P, D).transpose(1, 0, 2).reshape(P, -1))
        # query image: col pos*DN*QB + d*QB + q  <-  ctxT[d*P+p, block_q]
        ctxQ = np.concatenate(
            [ctx_bT[:, o * QB:(o + 1) * QB] for o in order], axis=1)  # [D, QTOT]
        ctq_img = np.ascontiguousarray(
            ctxQ.reshape(DN, P, NPOSL, QB).transpose(1, 2, 0, 3).reshape(P, -1))
        qpos_row = np.concatenate(
            [np.arange(o * QB, (o + 1) * QB) for o in order]).astype(np.float32)
        qpos_b = np.broadcast_to(qpos_row, (P, QTOT))
        a_full = (ctx_b @ wkbq).astype(np.float32)  # [N]
        abk = a_full.reshape(NT, P).T
        cst = np.ascontiguousarray(  # qpos | kpos | abk | bvb
            np.concatenate([qpos_b, kpos, abk, bvb], axis=1, dtype=np.float32))
        in_maps.append({
            "ctk_d": ctk_img, "ctr_d": ctr_img, "ctq_d": ctq_img,
            "wkq_d": wkq_img, "wv_d": wv_img,
            "cstd": cst,
            "onesd": np.ones((P, 8), bf16),
        })
    return in_maps


def assemble(results, B, N, D):
    out = np.zeros((B, N, D), np.float32)
    for c, res in enumerate(results):
        b, j = divmod(c, 2)
        order = _block_order(j)
        o = np.asarray(res["out"], np.float32)
        for p, blk in enumerate(order):
            out[b, blk * QB:(blk + 1) * QB] = o[p * QB:(p + 1) * QB]
    return out


def run(inputs, trace=False, **spmd_kwargs):
    context = np.asarray(inputs["context"])
    B, N, D = context.shape
    has_bias = bool(np.any(np.asarray(inputs["b_qkv"])))
    nc = build(N, D, has_bias=has_bias)
    in_maps = make_in_maps(context, inputs["W_qkv"], inputs["b_qkv"], n_cores=8)
    res = run_bass_kernel_spmd(nc, in_maps, core_ids=list(range(8)), trace=trace, **spmd_kwargs)
    out = assemble(res.results, B, N, D)
    return out, res


def kernel(context, W_qkv, b_qkv):
    out, _ = run({"context": context, "W_qkv": W_qkv, "b_qkv": b_qkv})
    return out
